# revision 6
# baseline (speedup 1.0000x reference)
"""Multi-head attention Bass kernel for Trainium2, 8-core SPMD.

Problem: B=2, S=2048, H=1024, 16 heads of 64 (torch-style MHA without
1/sqrt(d) scaling, key-padding mask, eval mode).

Sharding: core c handles batch b = c//4 and 4 heads (feature slice
256*(c%4) .. +256). Each core computes Q/K/V projections for its feature
slice over its batch, then attention for its 4 heads, producing
out[b, :, fslice]. Host concatenates.

Device-side layout trick: everything runs "transposed" (feature dim on
partitions) so that the only data transposes needed are the input
activations (done on the PE with transpose-mode matmuls):
  - Q^T, K^T [f, s]: scores S^T[kpos, q] = K^T.T @ Q^T (contraction d=64)
  - mask bias is per-kpos = per-partition -> folded into the exp()
    activation's bias operand
  - V kept [s, f] with an appended ones column per head, so the PV matmul
    out[0:64,:] = unnormalized out^T and out[64,:] = softmax denominator
  - small PE transpose of the [65, q] result gives [q, 65] where the
    normalization (mult by reciprocal of col 64) is a natural
    per-partition tensor_scalar op.
No max-subtraction in softmax: |scores| <~ 50 for randn inputs, exp fits
fp32 comfortably (reference softmax subtracts max, mathematically equal).

Matmuls run as float32r (4-byte storage, 1 PE cycle/row vs 4 for fp32).
"""
import numpy as np

import concourse.bass as bass
import concourse.mybir as mybir
import concourse.tile as tile
from concourse.bass_utils import run_bass_kernel_spmd
from concourse.masks import make_identity

B, S, H = 2, 2048, 1024
NH, HD = 16, 64
N_CORES = 8
HPC = NH // (N_CORES // B)   # 4 heads per core
F = HPC * HD                 # 256 features per core
NEG = -10000000000.0
FP32R = True                 # matmuls in float32r (tf32-like) vs fp32

F32 = mybir.dt.float32
F32R = mybir.dt.float32r
MMDT = F32R if FP32R else F32


def _legalize_sync(nc, max_waits=1, max_updates=1):
    """This walrus build supports at most 1 sync wait / 1 sync update per
    instruction; split excess waits onto preceding same-engine NoOps."""
    n_upd = 0
    for f in nc.m.functions:
        for blk in f.blocks:
            out = []
            for inst in blk.instructions:
                si = getattr(inst, "sync_info", None)
                if si is not None and len(si.on_wait) > max_waits:
                    waits = list(si.on_wait)
                    for k, w in enumerate(waits[:-max_waits]):
                        out.append(mybir.InstNoOp(
                            name=f"{inst.name}-wsplit{k}",
                            sync_info=mybir.SyncInfo(on_wait=[w], on_update=[]),
                            bass_nofuse=True,
                            engine=inst.engine,
                        ))
                    inst.sync_info = mybir.SyncInfo(
                        on_wait=waits[-max_waits:], on_update=list(si.on_update))
                si = getattr(inst, "sync_info", None)
                if si is not None and len(si.on_update) > max_updates:
                    n_upd += 1
                out.append(inst)
            blk.instructions = out
    if n_upd:
        raise RuntimeError(f"{n_upd} instructions need >1 sync updates")


def _emit(nc, tc, d):
    """Emit the per-core program. d: dict of dram APs."""
    from contextlib import ExitStack
    Exp = mybir.ActivationFunctionType.Exp
    Ident = mybir.ActivationFunctionType.Identity
    NQ = S // 512     # 4 groups of 512 positions
    NT = S // 128     # 16 tiles of 128 positions
    VW = F + HPC      # 260: per kt-tile V row block incl. ones columns

    with ExitStack() as ctx:
        const = ctx.enter_context(tc.tile_pool(name="const", bufs=1))
        ident = const.tile([128, 128], F32, tag="ident")
        make_identity(nc, ident)
        bqk_sb = const.tile([128, 4], F32, tag="bqk")    # cols 0-1 bq, 2-3 bk
        nc.sync.dma_start(bqk_sb[:, 0:2], d["bqr"])
        nc.sync.dma_start(bqk_sb[:, 2:4], d["bkr"])
        mb_sb = const.tile([128, NT], F32, tag="mb")
        nc.sync.dma_start(mb_sb[:], d["mbias"])
        bvb = const.tile([128, F], F32, tag="bvb")
        nc.sync.dma_start(bvb[:], d["bvr"].to_broadcast((128, F)))

        qkv = ctx.enter_context(tc.tile_pool(name="qkv", bufs=1))
        QT = qkv.tile([128, 2 * S], MMDT, tag="QT")   # ptile m at cols 2048m
        KT = qkv.tile([128, 2 * S], MMDT, tag="KT")
        V = qkv.tile([128, NT * VW], MMDT, tag="V")   # kt at cols VW*kt
        outp = [qkv.tile([128, F], F32, tag=f"out{t}", name=f"out{t}")
                for t in range(NT)]
        nc.gpsimd.memset(V[:].bitcast(mybir.dt.int32), 0x3F800000)  # 1.0f; ones cols survive copies

        # ---- Phase B: load + transpose inputs, projections ----
        with ExitStack() as bctx:
            wT_p = bctx.enter_context(tc.tile_pool(name="wT", bufs=1))
            xin_p = bctx.enter_context(tc.tile_pool(name="xin", bufs=6))
            xT_p = bctx.enter_context(tc.tile_pool(name="xT", bufs=2))
            ps_tr = bctx.enter_context(
                tc.tile_pool(name="ps_tr", bufs=3, space="PSUM"))
            ps_qk = bctx.enter_context(
                tc.tile_pool(name="ps_qk", bufs=2, space="PSUM"))
            ps_v = bctx.enter_context(
                tc.tile_pool(name="ps_v", bufs=2, space="PSUM"))

            w_sb = {}
            for nm in ("wq", "wk", "wv"):
                w = wT_p.tile([128, 8 * F], MMDT, tag=nm)
                nc.sync.dma_start(
                    w[:].rearrange("p (c f) -> p c f", c=8),
                    d[nm + "T"].rearrange("(c p) f -> p c f", p=128))
                w_sb[nm] = w

            for nm, x_d in (("wq", d["xq"]), ("wk", d["xk"]), ("wv", d["xv"])):
                for g in range(NQ):
                    xT = xT_p.tile([128, 4096], MMDT)
                    for j in range(4):
                        xin = xin_p.tile([128, H], F32)
                        nc.sync.dma_start(
                            xin[:], x_d[512 * g + 128 * j:512 * g + 128 * (j + 1), :])
                        for c in range(8):
                            pt = ps_tr.tile([128, 128], F32)
                            nc.tensor.transpose(
                                pt[:], xin[:, 128 * c:128 * (c + 1)], ident[:])
                            nc.vector.tensor_copy(
                                xT[:, 512 * c + 128 * j:512 * c + 128 * (j + 1)],
                                pt[:])
                    if nm != "wv":
                        for m in range(2):
                            pq = ps_qk.tile([128, 512], F32)
                            for c in range(8):
                                nc.tensor.matmul(
                                    pq[:],
                                    (w_sb[nm][:, 256 * c + 128 * m:
                                                256 * c + 128 * (m + 1)]),
                                    (xT[:, 512 * c:512 * (c + 1)]),
                                    start=(c == 0), stop=(c == 7))
                            dst = QT if nm == "wq" else KT
                            col = m if nm == "wq" else 2 + m
                            nc.scalar.activation(
                                dst[:, 2048 * m + 512 * g:2048 * m + 512 * (g + 1)],
                                pq[:], Ident, bias=bqk_sb[:, col:col + 1])
                    else:
                        for j in range(4):
                            pv = ps_v.tile([128, F], F32)
                            for c in range(8):
                                nc.tensor.matmul(
                                    pv[:],
                                    (xT[:, 512 * c + 128 * j:
                                          512 * c + 128 * (j + 1)]),
                                    (w_sb["wv"][:, 256 * c:256 * (c + 1)]),
                                    start=(c == 0), stop=(c == 7))
                            t = 4 * g + j
                            for h in range(HPC):
                                nc.vector.tensor_copy(
                                    V[:, VW * t + 65 * h:VW * t + 65 * h + 64],
                                    pv[:, 64 * h:64 * (h + 1)])

        # ---- Phase C: attention ----
        with ExitStack() as cctx:
            ps_s = cctx.enter_context(
                tc.tile_pool(name="ps_s", bufs=3, space="PSUM"))
            ps_o = cctx.enter_context(
                tc.tile_pool(name="ps_o", bufs=2, space="PSUM"))
            ps_t = cctx.enter_context(
                tc.tile_pool(name="ps_t", bufs=2, space="PSUM"))
            es_p = cctx.enter_context(tc.tile_pool(name="expS", bufs=4))
            oT_p = cctx.enter_context(tc.tile_pool(name="oT", bufs=2))
            sm_p = cctx.enter_context(tc.tile_pool(name="sm", bufs=4))

            for g in range(NQ):
                for h in range(HPC):
                    m, po = h // 2, 64 * (h % 2)
                    acc = ps_o.tile([128, 512], F32)
                    for kt in range(NT):
                        ps = ps_s.tile([128, 512], F32)
                        nc.tensor.matmul(
                            ps[:],
                            (KT[po:po + 64,
                                  2048 * m + 128 * kt:2048 * m + 128 * (kt + 1)]),
                            (QT[po:po + 64,
                                  2048 * m + 512 * g:2048 * m + 512 * (g + 1)]),
                            start=True, stop=True)
                        es = es_p.tile([128, 512], MMDT)
                        nc.scalar.activation(
                            es[:], ps[:], Exp, bias=mb_sb[:, kt:kt + 1])
                        nc.tensor.matmul(
                            acc[0:65, :],
                            (V[:, VW * kt + 65 * h:VW * kt + 65 * (h + 1)]),
                            (es[:]),
                            start=(kt == 0), stop=(kt == NT - 1))
                    oT = oT_p.tile([128, 512], F32)
                    nc.scalar.copy(oT[0:65, :], acc[0:65, :])
                    for j in range(4):
                        pt = ps_t.tile([128, 65], F32)
                        nc.tensor.transpose(
                            pt[:, 0:65], oT[0:65, 128 * j:128 * (j + 1)],
                            ident[0:65, 0:65])
                        rc = sm_p.tile([128, 1], F32, tag="rc")
                        nc.vector.reciprocal(rc[:], pt[:, 64:65])
                        tmp = sm_p.tile([128, 64], F32, tag="tmp")
                        nc.vector.tensor_scalar_mul(tmp[:], pt[:, 0:64], rc[:])
                        nc.vector.tensor_add(
                            outp[4 * g + j][:, 64 * h:64 * (h + 1)],
                            tmp[:], bvb[:, 64 * h:64 * (h + 1)])
                for j in range(4):
                    nc.sync.dma_start(
                        d["out"][512 * g + 128 * j:512 * g + 128 * (j + 1), :],
                        outp[4 * g + j][:])


_NC_CACHE = None


def _build():
    global _NC_CACHE
    if _NC_CACHE is not None:
        return _NC_CACHE
    nc = bass.Bass(trn_type="TRN2", target_bir_lowering=False, debug=False)
    d = {
        "xq": nc.dram_tensor("xq", [S, H], F32, kind="ExternalInput").ap(),
        "xk": nc.dram_tensor("xk", [S, H], F32, kind="ExternalInput").ap(),
        "xv": nc.dram_tensor("xv", [S, H], F32, kind="ExternalInput").ap(),
        "wqT": nc.dram_tensor("wqT", [H, F], MMDT, kind="ExternalInput").ap(),
        "wkT": nc.dram_tensor("wkT", [H, F], MMDT, kind="ExternalInput").ap(),
        "wvT": nc.dram_tensor("wvT", [H, F], MMDT, kind="ExternalInput").ap(),
        "bqr": nc.dram_tensor("bqr", [128, 2], F32, kind="ExternalInput").ap(),
        "bkr": nc.dram_tensor("bkr", [128, 2], F32, kind="ExternalInput").ap(),
        "bvr": nc.dram_tensor("bvr", [1, F], F32, kind="ExternalInput").ap(),
        "mbias": nc.dram_tensor("mbias", [128, S // 128], F32,
                                kind="ExternalInput").ap(),
        "out": nc.dram_tensor("out", [S, F], F32, kind="ExternalOutput").ap(),
    }
    with tile.TileContext(nc) as tc:
        _emit(nc, tc, d)
    _legalize_sync(nc)
    _NC_CACHE = nc
    return nc


def make_in_maps(query, key, value, mask, Wq, bq, Wk, bk, Wv, bv):
    query, key, value = (np.asarray(a, np.float32) for a in (query, key, value))
    Wq, Wk, Wv = (np.asarray(a, np.float32) for a in (Wq, Wk, Wv))
    bq, bk, bv = (np.asarray(a, np.float32) for a in (bq, bk, bv))
    mask = np.asarray(mask)
    in_maps = []
    for c in range(N_CORES):
        b = c // (N_CORES // B)
        fi = c % (N_CORES // B)
        fs = F * fi
        mb = np.where(mask[b] == 0, np.float32(NEG), np.float32(0.0))
        in_maps.append({
            "xq": np.ascontiguousarray(query[b]),
            "xk": np.ascontiguousarray(key[b]),
            "xv": np.ascontiguousarray(value[b]),
            "wqT": np.ascontiguousarray(Wq[fs:fs + F].T),
            "wkT": np.ascontiguousarray(Wk[fs:fs + F].T),
            "wvT": np.ascontiguousarray(Wv[fs:fs + F].T),
            "bqr": np.ascontiguousarray(bq[fs:fs + F].reshape(2, 128).T),
            "bkr": np.ascontiguousarray(bk[fs:fs + F].reshape(2, 128).T),
            "bvr": np.ascontiguousarray(bv[fs:fs + F].reshape(1, F)),
            "mbias": np.ascontiguousarray(
                mb.astype(np.float32).reshape(S // 128, 128).T),
        })
    return in_maps


def assemble(results):
    out = np.empty((B, S, H), np.float32)
    for c in range(N_CORES):
        b = c // (N_CORES // B)
        fs = F * (c % (N_CORES // B))
        out[b, :, fs:fs + F] = results[c]["out"]
    return out


def kernel(query, key, value, mask, Wq, bq, Wk, bk, Wv, bv, _trace=False):
    nc = _build()
    in_maps = make_in_maps(query, key, value, mask, Wq, bq, Wk, bk, Wv, bv)
    res = run_bass_kernel_spmd(nc, in_maps, core_ids=list(range(N_CORES)),
                               trace=_trace)
    out = assemble(res.results)
    if _trace:
        return out, res
    return out


# revision 10
# speedup vs baseline: 1.0236x; 1.0236x over previous
"""Multi-head attention Bass kernel for Trainium2, 8-core SPMD.

Problem: B=2, S=2048, H=1024, 16 heads of 64 (torch-style MHA without
1/sqrt(d) scaling, key-padding mask, eval mode).

Sharding: core c handles batch b = c//4 and 4 heads (feature slice
256*(c%4) .. +256). Each core computes Q/K/V projections for its feature
slice over its batch, then attention for its 4 heads, producing
out[b, :, fslice]. Host concatenates.

Key-padding compaction: masked key positions contribute exactly
exp(-1e10) = 0 to softmax, so the host drops masked key/value rows and
pads to a multiple of 256 (typically 1280 of 2048 remain). Padding rows
get the -1e10 bias so they also contribute 0. Numerically identical to
the reference up to fp summation order.

Device-side layout: everything runs "transposed" (feature dim on
partitions) so the only data transposes needed are the input activations
(PE transpose-mode matmuls):
  - Q^T, K^T [f, s]: scores S^T[kpos, q] = K^T.T @ Q^T (contraction d=64)
  - key-padding bias is per-kpos = per-partition -> folded into the
    exp() activation's bias operand
  - V kept [s, f] with an appended ones column per head, so the PV
    matmul gives out[0:64,:] = unnormalized out^T and out[64,:] = the
    softmax denominator
  - a small PE transpose of the [65, q] result gives [q, 65] where
    normalization (multiply by reciprocal of col 64) is a natural
    per-partition tensor_scalar op.
No max-subtraction in softmax: |scores| <~ 50 for randn-scale inputs,
exp fits fp32 comfortably (reference subtracts max; mathematically the
same ratio).

Matmuls and transposes run as float32r (4-byte storage, 1 PE cycle/row
vs 4 for fp32; transposes 1.5 vs 2).
"""
import numpy as np

import concourse.bass as bass
import concourse.mybir as mybir
import concourse.tile as tile
from concourse.bass_utils import run_bass_kernel_spmd
from concourse.masks import make_identity

B, S, H = 2, 2048, 1024
NH, HD = 16, 64
N_CORES = 8
HPC = NH // (N_CORES // B)   # 4 heads per core
F = HPC * HD                 # 256 features per core
NEG = -10000000000.0

F32 = mybir.dt.float32
F32R = mybir.dt.float32r
MMDT = F32R


def _legalize_sync(nc, max_waits=1, max_updates=1):
    """This walrus build supports at most 1 sync wait / 1 sync update per
    instruction; split excess waits onto preceding same-engine NoOps."""
    n_upd = 0
    for f in nc.m.functions:
        for blk in f.blocks:
            out = []
            for inst in blk.instructions:
                si = getattr(inst, "sync_info", None)
                if si is not None and len(si.on_wait) > max_waits:
                    waits = list(si.on_wait)
                    for k, w in enumerate(waits[:-max_waits]):
                        out.append(mybir.InstNoOp(
                            name=f"{inst.name}-wsplit{k}",
                            sync_info=mybir.SyncInfo(on_wait=[w], on_update=[]),
                            bass_nofuse=True,
                            engine=inst.engine,
                        ))
                    inst.sync_info = mybir.SyncInfo(
                        on_wait=waits[-max_waits:], on_update=list(si.on_update))
                si = getattr(inst, "sync_info", None)
                if si is not None and len(si.on_update) > max_updates:
                    n_upd += 1
                out.append(inst)
            blk.instructions = out
    if n_upd:
        raise RuntimeError(f"{n_upd} instructions need >1 sync updates")


def _groups(total):
    """Split `total` positions into DMA/proj groups of <=512 (multiples
    of 256 so float32r matmuls stay at full rate)."""
    out = []
    pos = 0
    while pos < total:
        w = min(512, total - pos)
        out.append((pos, w))
        pos += w
    return out


def _emit(nc, tc, d, s_kv):
    from contextlib import ExitStack
    Exp = mybir.ActivationFunctionType.Exp
    Ident = mybir.ActivationFunctionType.Identity
    NQ = S // 512        # 4 query groups of 512
    NTQ = S // 128       # 16 query tiles of 128
    NTK = s_kv // 128    # key tiles of 128
    VW = F + HPC         # 260: V row-block width incl. ones columns

    with ExitStack() as ctx:
        const = ctx.enter_context(tc.tile_pool(name="const", bufs=1))
        ident32 = const.tile([128, 128], F32, tag="ident32", name="ident32")
        make_identity(nc, ident32)
        ident = const.tile([128, 128], MMDT, tag="ident", name="ident")
        nc.vector.tensor_copy(ident[:], ident32[:])
        bqk_sb = const.tile([128, 4], F32, tag="bqk", name="bqk")
        nc.gpsimd.dma_start(bqk_sb[:, 0:2], d["bqr"])
        nc.gpsimd.dma_start(bqk_sb[:, 2:4], d["bkr"])
        mb_sb = const.tile([128, NTK], F32, tag="mb", name="mb")
        nc.gpsimd.dma_start(mb_sb[:], d["mbias"])
        bvb = const.tile([128, F], F32, tag="bvb", name="bvb")
        nc.gpsimd.dma_start(bvb[:], d["bvr"].to_broadcast((128, F)))

        qkv = ctx.enter_context(tc.tile_pool(name="qkv", bufs=1))
        QT = qkv.tile([128, 2 * S], MMDT, tag="QT", name="QT")
        KT = qkv.tile([128, 2 * s_kv], MMDT, tag="KT", name="KT")
        V = qkv.tile([128, NTK * VW], MMDT, tag="V", name="V")
        outp = [qkv.tile([128, F], F32, tag=f"out{t}", name=f"out{t}")
                for t in range(NTQ)]
        nc.gpsimd.memset(V[:].bitcast(mybir.dt.int32), 0x3F800000)  # 1.0f

        # ---- Phase B: load + transpose inputs, projections ----
        with ExitStack() as bctx:
            wT_p = bctx.enter_context(tc.tile_pool(name="wT", bufs=1))
            xin_p = bctx.enter_context(tc.tile_pool(name="xin", bufs=6))
            xT_p = bctx.enter_context(tc.tile_pool(name="xT", bufs=2))
            ps_tr = bctx.enter_context(
                tc.tile_pool(name="ps_tr", bufs=3, space="PSUM"))
            ps_qk = bctx.enter_context(
                tc.tile_pool(name="ps_qk", bufs=2, space="PSUM"))
            ps_v = bctx.enter_context(
                tc.tile_pool(name="ps_v", bufs=2, space="PSUM"))

            w_sb = {}
            for nm in ("wq", "wk", "wv"):
                w = wT_p.tile([128, 8 * F], MMDT, tag=nm, name=nm)
                nc.gpsimd.dma_start(
                    w[:].rearrange("p (c f) -> p c f", c=8),
                    d[nm + "T"].rearrange("(c p) f -> p c f", p=128))
                w_sb[nm] = w

            for nm, x_d, slen in (("wq", d["xq"], S), ("wk", d["xk"], s_kv),
                                  ("wv", d["xv"], s_kv)):
                for gpos, gw in _groups(slen):
                    nj = gw // 128
                    xT = xT_p.tile([128, 8 * 512], MMDT, tag="xT", name="xT")
                    xTv = xT[:, 0:8 * gw].rearrange("p (c b) -> p c b", c=8)
                    for j in range(nj):
                        xin = xin_p.tile([128, H], MMDT, tag="xin", name="xin")
                        nc.sync.dma_start(
                            xin[:], x_d[gpos + 128 * j:gpos + 128 * (j + 1), :])
                        for c0 in (0, 4):
                            pt = ps_tr.tile([128, 512], MMDT, tag="pt", name="pt")
                            for c in range(4):
                                nc.tensor.transpose(
                                    pt[:, 128 * c:128 * (c + 1)],
                                    xin[:, 128 * (c0 + c):128 * (c0 + c + 1)],
                                    ident[:])
                            nc.vector.tensor_copy(
                                xTv[:, c0:c0 + 4, 128 * j:128 * (j + 1)],
                                pt[:].rearrange("p (c b) -> p c b", c=4))
                    if nm != "wv":
                        dst = QT if nm == "wq" else KT
                        for m in range(2):
                            pq = ps_qk.tile([128, 512], F32, tag="pq", name="pq")
                            for c in range(8):
                                nc.tensor.matmul(
                                    pq[:, 0:gw],
                                    w_sb[nm][:, 256 * c + 128 * m:
                                             256 * c + 128 * (m + 1)],
                                    xTv[:, c, :],
                                    start=(c == 0), stop=(c == 7))
                            col = m if nm == "wq" else 2 + m
                            nc.scalar.activation(
                                dst[:, slen * m + gpos:slen * m + gpos + gw],
                                pq[:, 0:gw], Ident, bias=bqk_sb[:, col:col + 1])
                    else:
                        for j in range(nj):
                            pv = ps_v.tile([128, F], F32, tag="pv", name="pv")
                            for c in range(8):
                                nc.tensor.matmul(
                                    pv[:],
                                    xTv[:, c, 128 * j:128 * (j + 1)],
                                    w_sb["wv"][:, 256 * c:256 * (c + 1)],
                                    start=(c == 0), stop=(c == 7))
                            t = (gpos // 128) + j
                            nc.vector.tensor_copy(
                                V[:, VW * t:VW * (t + 1)]
                                .rearrange("p (h e) -> p h e", e=65)[:, :, 0:64],
                                pv[:].rearrange("p (h e) -> p h e", h=HPC))

        # ---- Phase C: attention ----
        with ExitStack() as cctx:
            ps_s = cctx.enter_context(
                tc.tile_pool(name="ps_s", bufs=3, space="PSUM"))
            ps_o = cctx.enter_context(
                tc.tile_pool(name="ps_o", bufs=2, space="PSUM"))
            ps_t = cctx.enter_context(
                tc.tile_pool(name="ps_t", bufs=2, space="PSUM"))
            es_p = cctx.enter_context(tc.tile_pool(name="expS", bufs=4))
            oT_p = cctx.enter_context(tc.tile_pool(name="oT", bufs=2))
            sm_p = cctx.enter_context(tc.tile_pool(name="sm", bufs=4))

            for g in range(NQ):
                for h in range(HPC):
                    m, po = h // 2, 64 * (h % 2)
                    acc = ps_o.tile([128, 512], F32, tag="acc", name="acc")
                    for kt in range(NTK):
                        ps = ps_s.tile([128, 512], F32, tag="ps", name="ps")
                        nc.tensor.matmul(
                            ps[:],
                            KT[po:po + 64,
                               s_kv * m + 128 * kt:s_kv * m + 128 * (kt + 1)],
                            QT[po:po + 64,
                               S * m + 512 * g:S * m + 512 * (g + 1)],
                            start=True, stop=True)
                        es = es_p.tile([128, 512], MMDT, tag="es", name="es")
                        nc.scalar.activation(
                            es[:], ps[:], Exp, bias=mb_sb[:, kt:kt + 1])
                        nc.tensor.matmul(
                            acc[0:65, :],
                            V[:, VW * kt + 65 * h:VW * kt + 65 * (h + 1)],
                            es[:],
                            start=(kt == 0), stop=(kt == NTK - 1))
                    oT = oT_p.tile([128, 512], F32, tag="oT", name="oT")
                    nc.scalar.copy(oT[0:65, :], acc[0:65, :])
                    for j in range(4):
                        pt = ps_t.tile([128, 65], F32, tag="ptt", name="ptt")
                        nc.tensor.transpose(
                            pt[:, 0:65], oT[0:65, 128 * j:128 * (j + 1)],
                            ident32[0:65, 0:65])
                        rc = sm_p.tile([128, 1], F32, tag="rc", name="rc")
                        nc.vector.reciprocal(rc[:], pt[:, 64:65])
                        tmp = sm_p.tile([128, 64], F32, tag="tmp", name="tmp")
                        nc.vector.tensor_scalar_mul(tmp[:], pt[:, 0:64], rc[:])
                        nc.vector.tensor_add(
                            outp[4 * g + j][:, 64 * h:64 * (h + 1)],
                            tmp[:], bvb[:, 64 * h:64 * (h + 1)])
                for j in range(4):
                    nc.sync.dma_start(
                        d["out"][512 * g + 128 * j:512 * g + 128 * (j + 1), :],
                        outp[4 * g + j][:])


_NC_CACHE = {}


def _build(s_kv):
    if s_kv in _NC_CACHE:
        return _NC_CACHE[s_kv]
    nc = bass.Bass(trn_type="TRN2", target_bir_lowering=False, debug=False)
    d = {
        "xq": nc.dram_tensor("xq", [S, H], MMDT, kind="ExternalInput").ap(),
        "xk": nc.dram_tensor("xk", [s_kv, H], MMDT, kind="ExternalInput").ap(),
        "xv": nc.dram_tensor("xv", [s_kv, H], MMDT, kind="ExternalInput").ap(),
        "wqT": nc.dram_tensor("wqT", [H, F], MMDT, kind="ExternalInput").ap(),
        "wkT": nc.dram_tensor("wkT", [H, F], MMDT, kind="ExternalInput").ap(),
        "wvT": nc.dram_tensor("wvT", [H, F], MMDT, kind="ExternalInput").ap(),
        "bqr": nc.dram_tensor("bqr", [128, 2], F32, kind="ExternalInput").ap(),
        "bkr": nc.dram_tensor("bkr", [128, 2], F32, kind="ExternalInput").ap(),
        "bvr": nc.dram_tensor("bvr", [1, F], F32, kind="ExternalInput").ap(),
        "mbias": nc.dram_tensor("mbias", [128, s_kv // 128], F32,
                                kind="ExternalInput").ap(),
        "out": nc.dram_tensor("out", [S, F], F32, kind="ExternalOutput").ap(),
    }
    with tile.TileContext(nc) as tc:
        _emit(nc, tc, d, s_kv)
    _legalize_sync(nc)
    _NC_CACHE[s_kv] = nc
    return nc


def plan_kv(mask):
    """Per-batch compaction plan: indices of valid key positions and the
    padded kv length shared across batches."""
    mask = np.asarray(mask)
    idxs = [np.nonzero(mask[b])[0] for b in range(B)]
    nmax = max((len(i) for i in idxs), default=1)
    s_kv = min(S, max(256, -(-nmax // 256) * 256))
    return idxs, s_kv


def make_in_maps(query, key, value, mask, Wq, bq, Wk, bk, Wv, bv,
                 idxs=None, s_kv=None):
    if idxs is None:
        idxs, s_kv = plan_kv(mask)
    query, key, value = (np.asarray(a, np.float32) for a in (query, key, value))
    Wq, Wk, Wv = (np.asarray(a, np.float32) for a in (Wq, Wk, Wv))
    bq, bk, bv = (np.asarray(a, np.float32) for a in (bq, bk, bv))
    in_maps = []
    kc, vc, mbc = {}, {}, {}
    for b in range(B):
        idx = idxs[b]
        kcb = np.zeros((s_kv, H), np.float32)
        kcb[:len(idx)] = key[b][idx]
        vcb = np.zeros((s_kv, H), np.float32)
        vcb[:len(idx)] = value[b][idx]
        mb = np.full(s_kv, NEG, np.float32)
        mb[:len(idx)] = 0.0
        kc[b], vc[b] = kcb, vcb
        mbc[b] = np.ascontiguousarray(mb.reshape(s_kv // 128, 128).T)
    for c in range(N_CORES):
        b = c // (N_CORES // B)
        fs = F * (c % (N_CORES // B))
        in_maps.append({
            "xq": np.ascontiguousarray(query[b]),
            "xk": kc[b],
            "xv": vc[b],
            "wqT": np.ascontiguousarray(Wq[fs:fs + F].T),
            "wkT": np.ascontiguousarray(Wk[fs:fs + F].T),
            "wvT": np.ascontiguousarray(Wv[fs:fs + F].T),
            "bqr": np.ascontiguousarray(bq[fs:fs + F].reshape(2, 128).T),
            "bkr": np.ascontiguousarray(bk[fs:fs + F].reshape(2, 128).T),
            "bvr": np.ascontiguousarray(bv[fs:fs + F].reshape(1, F)),
            "mbias": mbc[b],
        })
    return in_maps


def assemble(results):
    out = np.empty((B, S, H), np.float32)
    for c in range(N_CORES):
        b = c // (N_CORES // B)
        fs = F * (c % (N_CORES // B))
        out[b, :, fs:fs + F] = results[c]["out"]
    return out


def kernel(query, key, value, mask, Wq, bq, Wk, bk, Wv, bv, _trace=False):
    idxs, s_kv = plan_kv(mask)
    nc = _build(s_kv)
    in_maps = make_in_maps(query, key, value, mask, Wq, bq, Wk, bk, Wv, bv,
                           idxs, s_kv)
    res = run_bass_kernel_spmd(nc, in_maps, core_ids=list(range(N_CORES)),
                               trace=_trace)
    out = assemble(res.results)
    if _trace:
        return out, res
    return out
